# revision 2
# baseline (speedup 1.0000x reference)
"""Trainium2 Bass kernel for the hard-triplet margin-ranking loss.

Layout work happens on the host; the device does the O(N^2 D) work: the
distance matmuls and the hardest-positive/negative mining reductions.

Host prep: L2-normalize exactly like the reference, scale by 16, quantize to
fp8e4 (5.7e-4 loss error, gate 2e-2), transpose to K-major, and CLASS-SORT
both anchors and opposite-half columns (the loss is a mean over anchors, so
both permutations are free).  Anchors are dealt round-robin to the 4 cores
of each half, so row tile r covers sorted ranks [512r, 512(r+1)) on every
core and its same-class columns sit in a ~540-column span near column 512r,
identical across cores (spans are compile-time constants; the program is
rebuilt if they change).  The host also emits the per-tile same-class mask
mq (160 where classes match -- fp8 values >= 256 are NaN on-device) so the
device never touches raw targets.

Device per core (512 anchors x 2048 opposite, all matmuls fp8 at 1 cyc/col):
  - 4 row tiles x 2 halves of [128,1024] f32 psum (3 rotating + 2 quarter
    slots), 4 feature matmuls per half plus a -4I @ mq matmul over only the
    span columns (mid- or end-of-accumulation-group), shifting same-class
    entries by -640 > the 512 dot range.
  - Hardest negative = row max of shifted pm (max-cross >= -256 always
    beats the shifted positives); hardest positive = row min over the span.
    ACT evacuates h0 halves to fp16 (DVE 4x-mode tensor_scalar takes the
    max); DVE tensor_scalar evacuates h1 halves with accum_out giving the
    max for free.  Tile 3 h1 runs as two [128,512] quarters so the
    post-last-matmul tail is one fused [128,512] DVE pass.
  - The min accumulation fuses the no-positive clamp (op0 = min with -384,
    the exact dist_ap = 0 point); the max combine pre-shifts by -640.000128
    so dist_ap and dist_an share one sqrt form.  The device ships the mined
    per-anchor stats ([128,8] f32 per core); the host applies the closed
    form dap = sqrt(-mn/128 - 3), dan = sqrt(-mxs/128 - 3) and the
    margin-relu mean over 4096 anchors (the heavy reductions all happened
    on-device).
  - PE p-state warm-up: K=1 dummy matmuls bridge the DMA fill so the real
    matmuls run at 2.4 GHz; DMA is split SP/HWDGE (features, in first-use
    order) and Pool/SWDGE (masks) so nothing gates the stream.
"""

import numpy as np

N, D = 4096, 256
HALF = N // 2
NCORES = 8
RPC = N // NCORES   # 512 anchors per core
RT = RPC // 128     # 4 row tiles
MARGIN = 0.3
EPS = 1e-6
SHIFT = 640.0       # = 4 * 160; same-class shift, > 512 = full pm range
MQVAL = 160.0       # exact in fp8e4 (values >= 256 are NaN on-device)
CLAMP = -128.0 * (2.0 * SHIFT / 256.0 - 2.0)  # = -384: dist_ap = 0 point

_CACHE = {}


def _build(spans, n_warm=9):
    """spans: per-tile (lo, hi) column spans (union over cores, compile-time
    constants) containing all same-class columns of that tile's anchors."""
    from contextlib import ExitStack

    import concourse.bacc as bacc
    import concourse.bass as bass
    import concourse.tile as tile
    from concourse import mybir

    f32 = mybir.dt.float32
    f16 = mybir.dt.float16
    f8 = mybir.dt.float8e4
    Alu = mybir.AluOpType
    Act = mybir.ActivationFunctionType
    ts = bass.ts

    nc = bacc.Bacc(
        "TRN2",
        target_bir_lowering=False,
        debug=False,
        enable_asserts=True,
        num_devices=NCORES,
    )
    # feat = [anchors | opposite]: one tensor so DMA #1 can carry all the
    # anchors plus opposite block 0 in a single transfer
    feat = nc.dram_tensor(
        "feat", [128, 2, RPC + HALF], f8, kind="ExternalInput"
    ).ap()
    mqd = [
        nc.dram_tensor(f"mq{r}", [128, spans[r][1] - spans[r][0]], f8,
                       kind="ExternalInput").ap()
        for r in range(RT)
    ]
    out = nc.dram_tensor("out", [128, 8], f32, kind="ExternalOutput").ap()

    with tile.TileContext(nc) as tc, ExitStack() as ctx:
        const = ctx.enter_context(tc.tile_pool(name="const", bufs=1))
        xin = ctx.enter_context(tc.tile_pool(name="xin", bufs=1))
        w16p = ctx.enter_context(tc.tile_pool(name="w16p", bufs=1))
        stat = ctx.enter_context(tc.tile_pool(name="stat", bufs=1))
        psum = ctx.enter_context(tc.tile_pool(name="psum", bufs=3, space="PSUM"))
        qsum = ctx.enter_context(tc.tile_pool(name="qsum", bufs=2, space="PSUM"))

        # ---- Pool queue: warm-tile memsets first (PE warm-up starts
        # ~0.7us), then the mask SWDGE gens with jneg between so the mask
        # transfers interleave into the HWDGE gaps ------------------------
        warmw = const.tile([1, 128], f16, tag="warmw")
        nc.vector.memset(warmw[:], 0.0)
        warmr = const.tile([1, 256], f16, tag="warmr")
        nc.vector.memset(warmr[:], 0.0)

        mq = []
        for r in range(RT):
            lo, hi = spans[r]
            m = xin.tile([128, hi - lo], f8, tag=f"mq{r}", name=f"mqt{r}")
            mq.append(m)
        # fx holds [anchors | opposite] like the dram tensor; loaded as
        # (anchors + opposite block 0), block 1, blocks 2-3
        fx = xin.tile([128, 2, RPC + HALF], f8, tag="fx")
        xbt = fx[:, :, 0:RPC]

        def xop(c, b):
            return fx[:, c, RPC + 512 * b : RPC + 512 * (b + 1)]

        # Pool/SWDGE in need order
        nc.gpsimd.dma_start(mq[0][:], mqd[0][:])
        # jneg2 = -4 * I so the fp8 mask (0/160) lands as a -640 shift
        jneg2 = const.tile([128, 128], f8, tag="jneg2")
        nc.gpsimd.memset(jneg2[:], 0.0)
        nc.gpsimd.affine_select(
            out=jneg2[:], in_=jneg2[:], compare_op=Alu.not_equal,
            fill=-4.0, base=0, pattern=[[-1, 128]], channel_multiplier=1,
        )
        nc.gpsimd.dma_start(mq[1][:], mqd[1][:])
        nc.gpsimd.dma_start(mq[2][:], mqd[2][:])
        nc.gpsimd.dma_start(mq[3][:], mqd[3][:])

        # SP/HWDGE: anchors + xo block 0, then block 1, then blocks 2-3
        nc.sync.dma_start(fx[:, :, 0:1024], feat[:, :, 0:1024])
        nc.sync.dma_start(fx[:, :, 1024:1536], feat[:, :, 1024:1536])
        nc.sync.dma_start(fx[:, :, 1536:2560], feat[:, :, 1536:2560])

        # ---- PE warm-up + ACT table warm -------------------------------
        wps = qsum.tile([128, 512], f32, tag="pq", name="wps")
        for _ in range(n_warm):
            nc.tensor.matmul(wps[:, 0:256], lhsT=warmw[:], rhs=warmr[:],
                             start=True, stop=True)
        ones = const.tile([128, 1], f32, tag="ones")
        nc.vector.memset(ones[:], 1.0)
        wsc = const.tile([128, 1], f32, tag="wsc")
        nc.scalar.activation(wsc[:], ones[:], Act.Copy)
        nc.scalar.activation(wsc[:], ones[:], Act.Sqrt)

        # span pieces split at psum-block boundaries
        pieces = []
        for r in range(RT):
            lo, hi = spans[r]
            ps = []
            while lo < hi:
                nxt = min(hi, (lo // 512 + 1) * 512)
                ps.append((lo, nxt))
                lo = nxt
            pieces.append(ps)

        # ---- stats + epilogue constants --------------------------------
        mxp = stat.tile([128, 9], f32, tag="mxp")  # per-half/quarter partials
        # mm8 holds [mn0 mn1 mxs0 mxs1 | mn2 mn3 mxs2 mxs3]; mn slots are
        # clamp-fused mins, mxs slots are maxes pre-shifted by -640.000128 so
        # one Sqrt(bias=-3) yields [dap0 dap1 sv0 sv1 | ...]
        mm8 = stat.tile([128, 8], f32, tag="mm8")
        MXSH = -(SHIFT + 128.0 * EPS)  # fold the sv bias into the max shift

        w16 = [
            w16p.tile([128, 2048], f16, tag=f"w16_{r}", name=f"w16_{r}")
            for r in range(RT)
        ]
        scr = w16p.tile([128, 2048], f16, tag="scr")

        # ---- main loop -------------------------------------------------
        # Halves (r,h) run 4 feature MMs + span-piece mask MMs into
        # [128,1024] psum; tile 3 h1 runs as two [128,512] quarters so the
        # post-last-matmul tail is a single fused [128,512] DVE pass.
        # Evac: DVE-fused (tensor_scalar evac + max accum) for (1,h1),
        # (2,h1), (3,q3); ACT elsewhere.

        def mm_group(dst, r, b):
            """One 512-col block: features c0/c1 + this tile's mask pieces;
            full-block masks run last (as the stop) so the mask data may
            arrive after the features."""
            lo, hi = spans[r]
            here = [p for p in pieces[r] if p[0] // 512 == b]
            mids = [p for p in here if p[1] - p[0] < 512]
            fulls = [p for p in here if p[1] - p[0] == 512]
            nc.tensor.matmul(
                dst,
                lhsT=xbt[:, 0, r * 128 : (r + 1) * 128],
                rhs=xop(0, b),
                start=True, stop=False,
            )
            for plo, phi in mids:
                nc.tensor.matmul(
                    dst[:, plo - 512 * b : phi - 512 * b],
                    lhsT=jneg2[:],
                    rhs=mq[r][:, plo - lo : phi - lo],
                    start=False, stop=False,
                )
            nc.tensor.matmul(
                dst,
                lhsT=xbt[:, 1, r * 128 : (r + 1) * 128],
                rhs=xop(1, b),
                start=False, stop=not fulls,
            )
            for plo, phi in fulls:
                nc.tensor.matmul(
                    dst,
                    lhsT=jneg2[:],
                    rhs=mq[r][:, plo - lo : phi - lo],
                    start=False, stop=True,
                )

        def evac_fused(r, sl_w16, pm_ap, kslot):
            nc.vector.tensor_scalar(
                w16[r][:, sl_w16], pm_ap, 1.0, None,
                op0=Alu.mult, op1=Alu.max, accum_out=mxp[:, kslot : kslot + 1],
            )

        def evac_act(r, sl_w16, pm_ap, kslot):
            nc.scalar.copy(w16[r][:, sl_w16], pm_ap)
            nc.vector.tensor_scalar(
                scr[:, sl_w16], w16[r][:, sl_w16], 1.0, None,
                op0=Alu.mult, op1=Alu.max, accum_out=mxp[:, kslot : kslot + 1],
            )

        def finish_tile(r, kslots):
            lo, hi = spans[r]
            nc.vector.tensor_scalar(
                scr[:, 0 : len(kslots)],
                mxp[:, kslots[0] : kslots[0] + len(kslots)], MXSH, None,
                op0=Alu.add, op1=Alu.max,
                accum_out=mm8[:, 4 * (r // 2) + 2 + r % 2 :
                              4 * (r // 2) + 3 + r % 2],
            )
            nc.vector.tensor_scalar(
                scr[:, : hi - lo], w16[r][:, lo:hi], CLAMP, None,
                op0=Alu.min, op1=Alu.min,
                accum_out=mm8[:, 4 * (r // 2) + r % 2 :
                              4 * (r // 2) + 1 + r % 2],
            )

        # order: t0h0, t1h0, t0h1, t1h1, t2h0, t2h1, t3h0, quarters --
        # tile-1 h0 fills the wait for the xo blocks 2-3 DMA; ACT drains
        # h0 halves, DVE-fused drains h1 halves, tail is the q3 quarter
        def half(r, h, fused):
            pm = psum.tile([128, 1024], f32, tag="pm", name="pm")
            for bb in range(2):
                mm_group(pm[:, ts(bb, 512)], r, 2 * h + bb)
            if fused:
                evac_fused(r, slice(1024 * h, 1024 * h + 1024), pm[:], 2 * r + h)
            else:
                evac_act(r, slice(1024 * h, 1024 * h + 1024), pm[:], 2 * r + h)

        half(0, 0, False)
        half(0, 1, False)
        finish_tile(0, [0, 1])
        half(1, 0, False)
        half(1, 1, True)
        finish_tile(1, [2, 3])
        half(2, 0, False)
        half(2, 1, True)
        finish_tile(2, [4, 5])
        half(3, 0, False)
        pq2 = qsum.tile([128, 512], f32, tag="pq", name="pq2")
        mm_group(pq2[:], 3, 2)
        evac_act(3, slice(1024, 1536), pq2[:], 7)
        pq3 = qsum.tile([128, 512], f32, tag="pq", name="pq3")
        mm_group(pq3[:], 3, 3)
        evac_fused(3, slice(1536, 2048), pq3[:], 8)
        finish_tile(3, [6, 7, 8])
        nc.sync.dma_start(out[:], mm8[:])

    nc.compile()
    return nc


def _prep(inputs: np.ndarray, targets: np.ndarray):
    import ml_dtypes

    x = np.asarray(inputs, dtype=np.float32)
    t = np.asarray(targets, dtype=np.int64)
    xn = x / (np.linalg.norm(x, axis=1, keepdims=True) + EPS)
    q = (xn * 16.0).astype(ml_dtypes.float8_e4m3fn)

    spans = [[2048, 0] for _ in range(RT)]
    metas = []
    for c in range(NCORES):
        half = 0 if c < 4 else 1
        arows = np.arange(0, HALF) if half == 0 else np.arange(HALF, N)
        orows = np.arange(HALF, N) if half == 0 else np.arange(0, HALF)
        # interleave so tile r covers sorted ranks [512r, 512(r+1)) on every
        # core -> same-class columns land in the same span for all cores
        aorder = arows[np.argsort(t[arows], kind="stable")]
        mine = aorder[(c % 4)::4]
        oorder = orows[np.argsort(t[orows], kind="stable")]
        to_sorted = t[oorder]

        for r in range(RT):
            tcl = t[mine[r * 128 : (r + 1) * 128]]
            lo = int(np.searchsorted(to_sorted, tcl.min(), side="left"))
            hi = int(np.searchsorted(to_sorted, tcl.max(), side="right"))
            if hi > lo:
                spans[r][0] = min(spans[r][0], lo)
                spans[r][1] = max(spans[r][1], hi)
        # every anchor must have a cross-class column (reference would set
        # dist_an = 1.0 otherwise; impossible with 512 random classes)
        cnt = np.bincount(to_sorted, minlength=512)
        assert (cnt[t[mine]] < HALF).all()
        metas.append((mine, to_sorted, oorder))

    spans = tuple(
        (max(0, lo - lo % 16), min(HALF, hi + (-hi) % 16)) for lo, hi in spans
    )
    in_maps = []
    for c in range(NCORES):
        mine, to_sorted, oorder = metas[c]
        im = {
            "feat": np.ascontiguousarray(
                np.concatenate(
                    [
                        q[mine].reshape(RPC, 2, 128).transpose(2, 1, 0),
                        q[oorder].reshape(HALF, 2, 128).transpose(2, 1, 0),
                    ],
                    axis=2,
                )
            ),
        }
        for r in range(RT):
            lo, hi = spans[r]
            tcl = t[mine[r * 128 : (r + 1) * 128]]
            m = (to_sorted[None, lo:hi] == tcl[:, None]).astype(np.float32)
            im[f"mq{r}"] = np.ascontiguousarray(
                (m * MQVAL).astype(ml_dtypes.float8_e4m3fn)
            )
        in_maps.append(im)
    return in_maps, spans


def _get_nc(spans):
    if _CACHE.get("key") != spans:
        _CACHE["nc"] = _build(spans)
        _CACHE["key"] = spans
    return _CACHE["nc"]


def kernel(inputs: np.ndarray, targets: np.ndarray) -> np.ndarray:
    from concourse.bass_utils import run_bass_kernel_spmd

    in_maps, spans = _prep(inputs, targets)
    nc = _get_nc(spans)
    res = run_bass_kernel_spmd(nc, in_maps, list(range(NCORES)))
    total = 0.0
    for i in range(NCORES):
        o = np.asarray(res.results[i]["out"], dtype=np.float64)
        mn = o[:, [0, 1, 4, 5]]
        mxs = o[:, [2, 3, 6, 7]]
        dap = np.sqrt(-mn / 128.0 - 3.0)
        dan = np.sqrt(-mxs / 128.0 - 3.0)
        total += np.maximum(dap - dan + MARGIN, 0.0).sum()
    return np.float32(total / N)
